# revision 50
# baseline (speedup 1.0000x reference)
"""Trainium2 Bass kernel for the 2-layer GATv2 + MLP-head model (nn_GAT_21028159881586).

Strategy (8 NeuronCores, SPMD single NEFF):
  * Destination-block partitioning: global nodes split into 8 slices of 3750
    (padded to 3840 = 30 windows x 128 per core).  Core c owns all edges whose
    destination lands in its slice, so segment softmax + aggregation are
    core-local.
  * Per layer: node transforms xl/xr = x@W+b on the local slice, AllGather of
    xl over the 8 cores, then 30 windows of 128 destinations each.
  * Per window (T tiles of 128 edges, single SWDGE gather):
      - ONE dma_gather of xl rows in (edge, channel) layout; trailing pad
        indices are -1 so the Q7 desc-gen trims them (desc-gen on the gpsimd
        pair is the kernel bottleneck at ~4.6ns/descriptor).
      - one-hot scatter matrix s_t[e,d] (DVE is_equal vs iota) for the
        aggregation matmul, and its transpose s_tT[d,e] (is_equal against a
        partition-broadcast of the dst-rel row) for expanding the window's
        128 xr rows to per-edge xr via PE matmul.
      - m = g + xr_e (DVE, PSUM+SBUF), leaky-relu (ACT Prelu),
        logits = rowwise (m*att).sum (DVE mult + reduce), alpha = exp(logit)
        (ACT), gw = g*alpha (DVE), and one accumulating PE matmul per tile
        with lhsT=s_t and rhs=[gw | alpha] producing both the aggregation and
        the softmax denominators in PSUM.
      - Pad edges carry dst-rel = -1 which matches no one-hot row/col, so
        they contribute nothing (no bias masking needed).
  * Softmax max-subtraction is skipped (logits are O(1); exp cannot
    overflow; mathematically identical).
  * MLP head: batch rows assigned to the core owning their var node; the
    selected h2 rows are dma_gathered transposed; 3-layer MLP runs
    transposed.

Everything runs in fp16 with fp32 PSUM accumulation.
"""

import numpy as np

import concourse.bacc as bacc
import concourse.tile as tile
import concourse.mybir as mybir
from concourse.bass_utils import run_bass_kernel_spmd

P = 128
NCORES = 8
N = 30000
NLOC_REAL = 3750          # real nodes per core
WIN = 30                  # destination windows per core
NLOC = WIN * P            # 3840 padded nodes per core
NALL = NCORES * NLOC      # 30720 padded global nodes
IN_DIM = 1281
KCH = 11                  # input-dim chunks of 128
KPAD = KCH * P            # 1408
HID = 256
HEADS1 = 4
BLOC = 640                # padded batch rows per core (actual max ~554)
NEG = 0.2
GE = 4                    # expansion-psum tiles per group (2 PSUM banks)
AGW = [30]                # AllGather chunk sizes in windows (single-writer rule
AGB = [0, 30]             # forbids >1 collective into one Shared tensor)
AGR = [w * P for w in AGB]            # chunk row boundaries (local)
AGOFF = [0]                           # global row offset of each chunk block
for _w in AGW:
    AGOFF.append(AGOFF[-1] + NCORES * _w * P)

PB = False                # build s_tT via partition-broadcast AP on DVE
TRIM = False              # -1-pad trimming of gather idxs

f32 = mybir.dt.float32
f16 = mybir.dt.float16
i16 = mybir.dt.int16
AF = mybir.ActivationFunctionType
OP = mybir.AluOpType

_nc_cache = {}


def _wrap16(idx2d: np.ndarray) -> np.ndarray:
    """(W, E) int -> (W*128, E//16) int16, wrapped in 16 partitions, replicated
    across the 8 gpsimd cores."""
    w, e = idx2d.shape
    assert e % 16 == 0
    t = idx2d.reshape(w, e // 16, 16).transpose(0, 2, 1)       # (W, 16, E/16)
    return np.tile(t, (1, 8, 1)).reshape(w * P, e // 16).astype(np.int16)


def _etile(v2d: np.ndarray) -> np.ndarray:
    """(W, E) -> (W*128, T) with [w*128+p, t] = v[w, t*128+p] (the layout of a
    transpose=False gather / per-tile PSUM partitions)."""
    w, e = v2d.shape
    t = v2d.reshape(w, e // P, P).transpose(0, 2, 1)           # (W, 128, T)
    return t.reshape(w * P, e // P)


def _preprocess(inputs):
    x = np.asarray(inputs["x"], np.float32)
    ei = np.asarray(inputs["edge_index"]).astype(np.int64)
    var_idx = np.asarray(inputs["var_node_idx"]).astype(np.int64)
    wt = np.asarray(inputs["wt_onehot"], np.float32)
    mut = np.asarray(inputs["mut_onehot"], np.float32)

    # self-loops are handled by a separate local path in the kernel (no
    # gather needed), so the edge lists carry only the real edges
    src = ei[0]
    dst = ei[1]

    # ---- degree-balanced window packing: per core, assign local nodes to
    # (window, slot) with LPT so every window has a near-equal edge count.
    import heapq
    perm_glob = np.zeros(N, np.int64)   # old global id -> core*NLOC + new local
    perm_tab = np.zeros(N, np.int64)    # old global id -> gather-table row
    perms = []
    dst_core_all = dst // NLOC_REAL
    dst_loc_all = dst - dst_core_all * NLOC_REAL
    for c in range(NCORES):
        deg = np.bincount(dst_loc_all[dst_core_all == c], minlength=NLOC_REAL)
        order_d = np.argsort(-deg, kind="stable")
        slots = [0] * WIN
        load = [0] * WIN
        perm = np.zeros(NLOC_REAL, np.int64)
        h = [(0, w) for w in range(WIN)]
        heapq.heapify(h)
        for v in order_d:
            while True:
                l, w = heapq.heappop(h)
                if slots[w] < P:
                    break
            perm[v] = w * P + slots[w]
            slots[w] += 1
            load[w] += int(deg[v])
            if slots[w] < P:
                heapq.heappush(h, (load[w], w))
        perms.append(perm)
        perm_glob[c * NLOC_REAL:(c + 1) * NLOC_REAL] = c * NLOC + perm
        # gather-table row under the block-chunked AllGather layout:
        # rows grouped as [chunk k][core r][local rows of chunk k]
        kk = np.searchsorted(np.asarray(AGR[1:]), perm, side="right")
        goff = np.asarray(AGOFF[:-1])[kk]
        szs = np.asarray([w * P for w in AGW])[kk]
        base = np.asarray(AGR[:-1])[kk]
        perm_tab[c * NLOC_REAL:(c + 1) * NLOC_REAL] = goff + c * szs + (perm - base)

    src_pad = perm_tab[src]
    dst_new = perm_glob[dst]

    order = np.argsort(dst_new, kind="stable")
    src_pad = src_pad[order]
    dst_s = dst_new[order]

    core_of = dst_s // NLOC
    dloc = dst_s - core_of * NLOC                           # local dst 0..3839
    win_of = dloc // P

    # max edges in any (core, window) -> uniform padded window size (x128)
    flat = core_of * WIN + win_of
    counts = np.bincount(flat, minlength=NCORES * WIN)
    ew = int(((counts.max() + P - 1) // P) * P)

    per_core = []
    for c in range(NCORES):
        sel = core_of == c
        sp_c, dl_c, w_c = src_pad[sel], dloc[sel], win_of[sel]
        srcw = np.zeros((WIN, ew), np.int64)
        drel = np.full((WIN, ew), -1.0, np.float32)
        for w in range(WIN):
            m = w_c == w
            k = int(m.sum())
            # order the window's edges by source for HBM locality
            o = np.argsort(sp_c[m], kind="stable")
            srcw[w, :k] = sp_c[m][o]
            if TRIM and w >= 2:
                # trailing -1 idxs are trimmed by the Q7 desc-gen; windows
                # 0/1 gather row 0 instead so the rotating SBUF buffers are
                # fully initialized with finite data.
                srcw[w, k:] = -1
            drel[w, :k] = (dl_c[m][o] - w * P).astype(np.float32)
        per_core.append(dict(
            si=_wrap16(srcw),
            drt=_etile(drel).astype(np.float32),
            drw=drel.astype(np.float16),
        ))

    # ---- shared weights / constants
    def pad_kT(w, m):  # (IN_DIM, m) -> (128, KCH*m) f16 chunked layout
        wp = np.zeros((KPAD, m), np.float32)
        wp[:IN_DIM] = w
        return wp.reshape(KCH, P, m).transpose(1, 0, 2).reshape(P, KCH * m).astype(np.float16)

    def two_chunk(w):  # (256, M) -> (128, 2*M) f16
        m = w.shape[1]
        return w.reshape(2, P, m).transpose(1, 0, 2).reshape(P, 2 * m).astype(np.float16)

    def rep_bias(b):  # (HID,) -> (128, HID) f32
        return np.broadcast_to(np.asarray(b, np.float32)[None, :], (P, HID)).copy()

    att1 = np.asarray(inputs["att1"], np.float32).reshape(1, HID)   # (4,64)->(1,256)
    att2 = np.asarray(inputs["att2"], np.float32).reshape(1, HID)

    hW1 = np.asarray(inputs["hW1"], np.float32)             # (296, 128)
    wlr1 = np.concatenate([np.asarray(inputs["Wl1"], np.float32),
                           np.asarray(inputs["Wr1"], np.float32)], axis=1)
    wlr2 = np.concatenate([np.asarray(inputs["Wl2"], np.float32),
                           np.asarray(inputs["Wr2"], np.float32)], axis=1)
    shared = dict(
        wlr1=pad_kT(wlr1, 2 * HID),
        wlr2=two_chunk(wlr2),
        attb1=np.broadcast_to(att1, (P, HID)).astype(np.float16).copy(),
        attb2=np.broadcast_to(att2, (P, HID)).astype(np.float16).copy(),
        blr1=np.concatenate([rep_bias(inputs["bl1"]), rep_bias(inputs["br1"])], 1),
        bias1=rep_bias(inputs["bias1"]),
        blr2=np.concatenate([rep_bias(inputs["bl2"]), rep_bias(inputs["br2"])], 1),
        bias2=rep_bias(inputs["bias2"]),
        hw1a=hW1[0:128].astype(np.float16),
        hw1b=hW1[128:256].astype(np.float16),
        hw1c=np.vstack([hW1[256:296], np.zeros((8, 128), np.float32)]).astype(np.float16),
        hw2=np.asarray(inputs["hW2"], np.float32).astype(np.float16),   # (128, 64)
        hw3=np.asarray(inputs["hW3"], np.float32).astype(np.float16),   # (64, 1)
        hb1=np.asarray(inputs["hb1"], np.float32).reshape(P, 1),
        hb2=np.asarray(inputs["hb2"], np.float32).reshape(64, 1),
        hb3=np.asarray(inputs["hb3"], np.float32).reshape(1, 1),
        iota=np.broadcast_to(np.arange(P, dtype=np.float32)[None, :], (P, P)).copy(),
        iotap=np.arange(P, dtype=np.float32).reshape(P, 1).copy(),
        ones1=np.ones((1, P), np.float16),
        ident=np.eye(P, dtype=np.float16),
        zm1=np.concatenate([np.zeros((P, 1), np.float32),
                            np.full((P, 1), -1.0, np.float32)], axis=1),
    )

    # ---- per-core x slices, transposed + padded, chunked layout (128, KCH*NLOC)
    for c in range(NCORES):
        xp = np.zeros((KPAD, NLOC), np.float32)
        xp[:IN_DIM, perms[c]] = x[c * NLOC_REAL:(c + 1) * NLOC_REAL].T
        per_core[c]["xt"] = xp.reshape(KCH, P, NLOC).transpose(1, 0, 2).reshape(
            P, KCH * NLOC).astype(np.float16)

    # ---- MLP batch assignment: rows go to the core owning their var node
    vcore = var_idx // NLOC_REAL
    vloc = perm_glob[var_idx] - vcore * NLOC
    batch_rows = []
    for c in range(NCORES):
        rows = np.nonzero(vcore == c)[0]
        assert len(rows) <= BLOC, f"core {c} has {len(rows)} batch rows > {BLOC}"
        batch_rows.append(rows)
        vi = np.zeros((1, BLOC), np.int64)
        vi[0, :len(rows)] = vloc[rows]
        per_core[c]["varloc"] = _wrap16(vi)
        wm = np.zeros((40, BLOC), np.float32)
        wm[:20, :len(rows)] = wt[rows].T
        wm[20:, :len(rows)] = mut[rows].T
        per_core[c]["wtmut"] = wm.astype(np.float16)

    return per_core, shared, batch_rows, ew


def _build(ew, no_collectives=False):
    T = ew // P
    nc = bacc.Bacc("TRN2", target_bir_lowering=False, debug=False,
                   num_devices=1 if no_collectives else NCORES,
                   num_swdge_queues=1)

    # ---------- I/O ----------
    io = {}
    io["xt"] = nc.dram_tensor("xt", [P, KCH * NLOC], f16, kind="ExternalInput")
    for nm, sh, dt in (
        ("wlr1", [P, KCH * 2 * HID], f16), ("wlr2", [P, 4 * HID], f16),
        ("attb1", [P, HID], f16), ("attb2", [P, HID], f16),
        ("blr1", [P, 2 * HID], f32), ("bias1", [P, HID], f32),
        ("blr2", [P, 2 * HID], f32), ("bias2", [P, HID], f32),
        ("hw1a", [P, P], f16), ("hw1b", [P, P], f16), ("hw1c", [48, P], f16),
        ("hw2", [P, 64], f16), ("hw3", [64, 1], f16),
        ("hb1", [P, 1], f32), ("hb2", [64, 1], f32), ("hb3", [1, 1], f32),
        ("iota", [P, P], f32), ("iotap", [P, 1], f32), ("ones1", [1, P], f16),
        ("ident", [P, P], f16), ("zm1", [P, 2], f32),
        ("si", [WIN * P, ew // 16], i16),
        ("drt", [WIN * P, T], f32),
        ("drw", [WIN, ew], f16),
        ("varloc", [P, BLOC // 16], i16), ("wtmut", [40, BLOC], f16),
    ):
        io[nm] = nc.dram_tensor(nm, sh, dt, kind="ExternalInput")
    out = nc.dram_tensor("out", [1, BLOC], f32, kind="ExternalOutput")

    with tile.TileContext(nc) as tc:
        with (
            tc.tile_pool(name="const", bufs=1) as cp,
            tc.tile_pool(name="dram", bufs=1, space="DRAM") as dr,
        ):
            # resident constants
            c_ = {}
            for nm in ("wlr2", "attb1", "attb2", "bias1", "blr2", "bias2",
                       "hw1a", "hw1b", "hw1c", "hw2",
                       "hw3", "hb1", "hb2", "hb3", "iota", "iotap", "ones1",
                       "ident", "zm1", "varloc", "wtmut"):
                h = io[nm]
                c_[nm] = cp.tile(list(h.shape), h.dtype, tag=nm, name=f"c_{nm}")
                nc.sync.dma_start(c_[nm][:], h[:])

            # DRAM scratch
            xl1_loc = dr.tile([NLOC, HID], f16)
            xr1_loc = dr.tile([NLOC, HID], f16)
            xl1_all = dr.tile([NALL, HID], f16, addr_space="Shared")
            h1_loc = dr.tile([NLOC, HID], f16)
            xl2_loc = dr.tile([NLOC, HID], f16)
            xr2_loc = dr.tile([NLOC, HID], f16)
            xl2_all = dr.tile([NALL, HID], f16, addr_space="Shared")
            h2_loc = dr.tile([NLOC, HID], f16)

            # ---------- phase A layer 1 ----------
            with (
                tc.tile_pool(name="pa_sb", bufs=2) as sb,
                tc.tile_pool(name="pa_xt", bufs=1) as xp,
                tc.tile_pool(name="pa_ps", bufs=4, space="PSUM") as ps,
            ):
                xt = xp.tile([P, KCH, NLOC], f16)
                nc.sync.dma_start(xt[:], io["xt"][:].rearrange("p (k n) -> p k n", k=KCH))
                wlr1 = xp.tile([P, KCH, 2 * HID], f16)
                nc.sync.dma_start(wlr1[:], io["wlr1"][:].rearrange("p (k n) -> p k n", k=KCH))
                blr1 = xp.tile([P, 2 * HID], f32)
                nc.sync.dma_start(blr1[:], io["blr1"][:])
                for nt in range(WIN):
                    pa = ps.tile([P, 2 * HID], f32, tag="pa")
                    for k in range(KCH):
                        nc.tensor.matmul(pa[:], lhsT=xt[:, k, nt * P:(nt + 1) * P],
                                         rhs=wlr1[:, k, :],
                                         start=(k == 0), stop=(k == KCH - 1))
                    o = sb.tile([P, 2 * HID], f16, tag="pao")
                    nc.vector.tensor_tensor(out=o[:], in0=pa[:], in1=blr1[:],
                                            op=OP.add)
                    nc.scalar.dma_start(xl1_loc[nt * P:(nt + 1) * P, :], o[:, 0:HID])
                    nc.scalar.dma_start(xr1_loc[nt * P:(nt + 1) * P, :], o[:, HID:2 * HID])

            def emit_ag(xloc, xall):
                if no_collectives:
                    nc.sync.dma_start(xall[0:NLOC, :], xloc[:])
                else:
                    nc.gpsimd.collective_compute(
                        "AllGather", OP.bypass,
                        replica_groups=[list(range(NCORES))],
                        ins=[xloc[:].opt()], outs=[xall[:].opt()])

            emit_ag(xl1_loc, xl1_all)

            # ---------- layer 1 message passing ----------
            _emit_layer(nc, tc, ew=ew, heads=HEADS1, xl_all=xl1_all,
                        xl_loc=xl1_loc, xr_loc=xr1_loc, h_out=h1_loc,
                        att_bc=c_["attb1"], bias_mat=c_["bias1"], io=io,
                        c_=c_, tag="l1")

            # ---------- phase A layer 2 ----------
            # per-window chunks: each transpose depends only on its own h1
            # rows, so these DMAs drain during the tail of the L1 loop
            with (
                tc.tile_pool(name="pb_sb", bufs=3) as sb,
                tc.tile_pool(name="pb_ps", bufs=4, space="PSUM") as ps,
            ):
                blr2 = c_["blr2"]
                for nt in range(WIN):
                    rows = slice(nt * P, (nt + 1) * P)
                    htw = sb.tile([P, 2, P], f16, tag="htw", bufs=8)
                    for k in range(2):
                        nc.sync.dma_start_transpose(
                            htw[:, k, :], h1_loc[rows, k * P:(k + 1) * P])
                    pa = ps.tile([P, 2 * HID], f32, tag="pb")
                    for k in range(2):
                        nc.tensor.matmul(
                            pa[:], lhsT=htw[:, k, :],
                            rhs=c_["wlr2"][:, k * 2 * HID:(k + 1) * 2 * HID],
                            start=(k == 0), stop=(k == 1))
                    o = sb.tile([P, 2 * HID], f16, tag="pbo")
                    nc.vector.tensor_tensor(out=o[:], in0=pa[:], in1=blr2[:],
                                            op=OP.add)
                    nc.scalar.dma_start(xl2_loc[rows, :], o[:, 0:HID])
                    nc.scalar.dma_start(xr2_loc[rows, :], o[:, HID:2 * HID])

            emit_ag(xl2_loc, xl2_all)

            # ---------- layer 2 message passing ----------
            _emit_layer(nc, tc, ew=ew, heads=1, xl_all=xl2_all,
                        xl_loc=xl2_loc, xr_loc=xr2_loc, h_out=h2_loc,
                        att_bc=c_["attb2"], bias_mat=c_["bias2"], io=io,
                        c_=c_, tag="l2")

            # ---------- MLP head ----------
            with (
                tc.tile_pool(name="mlp_sb", bufs=2) as sb,
                tc.tile_pool(name="mlp_ps", bufs=2, space="PSUM") as ps,
            ):
                sel = sb.tile([P, 2, BLOC], f16)
                nc.gpsimd.dma_gather(sel[:], h2_loc[:], c_["varloc"][:],
                                     num_idxs=BLOC, num_idxs_reg=BLOC,
                                     elem_size=HID, transpose=True)
                for c0, cn in ((0, 512), (512, BLOC - 512)):
                    z1p = ps.tile([P, 512], f32, tag="z1p")
                    nc.tensor.matmul(z1p[:, :cn], lhsT=c_["hw1a"][:],
                                     rhs=sel[:, 0, c0:c0 + cn], start=True, stop=False)
                    nc.tensor.matmul(z1p[:, :cn], lhsT=c_["hw1b"][:],
                                     rhs=sel[:, 1, c0:c0 + cn], start=False, stop=False)
                    nc.tensor.matmul(z1p[:, :cn], lhsT=c_["hw1c"][0:40, :],
                                     rhs=c_["wtmut"][:, c0:c0 + cn], start=False, stop=True)
                    z1 = sb.tile([P, 512], f16, tag="z1")
                    nc.scalar.activation(z1[:, :cn], z1p[:, :cn], AF.Relu,
                                         bias=c_["hb1"][:])
                    z2p = ps.tile([64, 512], f32, tag="z2p")
                    nc.tensor.matmul(z2p[:, :cn], lhsT=c_["hw2"][:],
                                     rhs=z1[:, :cn], start=True, stop=True)
                    z2 = sb.tile([64, 512], f16, tag="z2")
                    nc.scalar.activation(z2[:, :cn], z2p[:, :cn], AF.Relu,
                                         bias=c_["hb2"][:])
                    z3p = ps.tile([1, 512], f32, tag="z3p")
                    nc.tensor.matmul(z3p[:, :cn], lhsT=c_["hw3"][:],
                                     rhs=z2[:, :cn], start=True, stop=True)
                    z3 = sb.tile([1, 512], f32, tag="z3")
                    nc.scalar.activation(z3[:, :cn], z3p[:, :cn], AF.Identity,
                                         bias=c_["hb3"][:])
                    nc.sync.dma_start(out[0:1, c0:c0 + cn], z3[:, :cn])

    nc.compile()
    return nc


def _emit_layer(nc, tc, *, ew, heads, xl_all, xl_loc, xr_loc, h_out, att_bc,
                bias_mat, io, c_, tag, post_win=None):
    T = ew // P
    CW = HID // heads
    NG = (T + GE - 1) // GE
    NCH = (T + 7) // 8        # gather chunks of up to 8 tiles (1024 idxs max)
    regs = {}
    for c in range(NCH):
        tn = min(8, T - c * 8)
        if tn * P not in regs:
            regs[tn * P] = nc.gpsimd.to_reg(tn * P)
    import contextlib
    with contextlib.ExitStack() as st:
        bigp = st.enter_context(tc.tile_pool(name=f"{tag}_big", bufs=2))
        sm = st.enter_context(tc.tile_pool(name=f"{tag}_sm", bufs=3))
        mpp = st.enter_context(tc.tile_pool(name=f"{tag}_mp", bufs=2, space="PSUM"))
        agp = st.enter_context(tc.tile_pool(name=f"{tag}_ag", bufs=2, space="PSUM"))
        bcp = st.enter_context(tc.tile_pool(name=f"{tag}_bc", bufs=2, space="PSUM"))

        def part_b(w, s_t, gwd, gws):
            _emit_part_b(nc, w=w, s_t=s_t, gwd=gwd, gws=gws, heads=heads,
                         h_out=h_out, bias_mat=bias_mat, c_=c_, sm=sm, agp=agp,
                         T=T, CW=CW, post_win=post_win)

        pend = None
        for w in range(WIN):
            rows = slice(w * P, (w + 1) * P)
            si = sm.tile([P, ew // 16], i16, tag="si")
            nc.sync.dma_start(si[:], io["si"][rows, :])
            drt = sm.tile([P, T], f32, tag="drt")
            nc.sync.dma_start(drt[:], io["drt"][rows, :])
            drw = sm.tile([1, ew], f16, tag="drw")
            nc.sync.dma_start(drw[:], io["drw"][w:w + 1, :])
            xrw = sm.tile([P, HID], f16, tag="xrw")
            nc.sync.dma_start(xrw[:], xr_loc[rows, :])
            xlw = sm.tile([P, HID], f16, tag="xlw")
            nc.sync.dma_start(xlw[:], xl_loc[rows, :])

            # SWDGE gathers: xl rows of all window edges, (e, c) layout
            g = bigp.tile([P, T, HID], f16, tag="g", bufs=4)
            for c in range(NCH):
                tt0 = c * 8
                tn = min(8, T - tt0)
                nc.gpsimd.dma_gather(g[:, tt0:tt0 + tn, :], xl_all[:],
                                     si[:, tt0 * 8:(tt0 + tn) * 8],
                                     num_idxs=tn * P, num_idxs_reg=regs[tn * P],
                                     elem_size=HID, transpose=False)

            # one-hot scatter matrices
            s_t = bigp.tile([P, T, P], f16, tag="s_t")
            nc.vector.tensor_tensor(
                out=s_t[:],
                in0=drt[:].unsqueeze(2).to_broadcast([P, T, P]),
                in1=c_["iota"][:].unsqueeze(1).to_broadcast([P, T, P]),
                op=OP.is_equal)
            s_tT = bigp.tile([P, T, P], f16, tag="s_tT")
            s_tT_flat = s_tT[:].rearrange("p t e -> p (t e)")
            for gi in range((ew + 511) // 512):
                c0 = gi * 512
                cn = min(512, ew - c0)
                bc = bcp.tile([P, 512], f32, tag="bc")
                nc.tensor.matmul(bc[:, 0:cn], lhsT=c_["ones1"][:],
                                 rhs=drw[0:1, c0:c0 + cn], start=True, stop=True)
                nc.vector.tensor_tensor(
                    out=s_tT_flat[:, c0:c0 + cn],
                    in0=bc[:, 0:cn],
                    in1=c_["iotap"][:].to_broadcast([P, cn]),
                    op=OP.is_equal)

            # expand xr to per-edge via PE, add g (identity matmul), leaky-relu
            lr = bigp.tile([P, T, HID], f16, tag="lr")
            for gi in range(NG):
                t0 = gi * GE
                gn = min(GE, T - t0)
                mp = mpp.tile([P, GE, HID], f32, tag="mp")
                for j in range(gn):
                    nc.tensor.matmul(mp[:, j, :], lhsT=s_tT[:, t0 + j, :],
                                     rhs=xrw[:], start=True, stop=False)
                    nc.tensor.matmul(mp[:, j, :], lhsT=c_["ident"][:],
                                     rhs=g[:, t0 + j, :], start=False, stop=True)
                nc.scalar.activation(lr[:, t0:t0 + gn, :], mp[:, 0:gn, :],
                                     AF.Prelu, alpha=NEG)

            # logits = per-head dot with att (att_bc rows are the flattened
            # per-head att vector, so no head split is needed for the product)
            law = bigp.tile([P, T, HID], f16, tag="law")
            nc.vector.tensor_tensor(
                out=law[:], in0=lr[:],
                in1=att_bc[:].unsqueeze(1).to_broadcast([P, T, HID]),
                op=OP.mult)
            logit = sm.tile([P, T * heads], f32, tag="lg")
            nc.vector.tensor_reduce(
                out=logit[:],
                in_=law[:].rearrange("p t (h c) -> p (t h) c", h=heads),
                axis=mybir.AxisListType.X, op=OP.add)

            # alpha = exp(logit) straight into the den columns of gwd
            gwd = bigp.tile([P, T, HID + heads], f16, tag="gwd")
            nc.scalar.activation(
                gwd[:, :, HID:HID + heads],
                logit[:].rearrange("p (t h) -> p t h", h=heads), AF.Exp)
            nc.vector.tensor_tensor(
                out=gwd[:, :, 0:HID].rearrange("p t (h c) -> p t h c", h=heads),
                in0=g[:].rearrange("p t (h c) -> p t h c", h=heads),
                in1=gwd[:, :, HID:HID + heads].unsqueeze(3)
                    .to_broadcast([P, T, heads, CW]),
                op=OP.mult)

            # self-loop path: m_self = xl[d] + xr[d], all local, scattered
            # into the aggregation via an identity matmul in part B
            ms = sm.tile([P, HID], f16, tag="ms")
            nc.vector.tensor_tensor(out=ms[:], in0=xlw[:], in1=xrw[:],
                                    op=OP.add)
            lrs = sm.tile([P, HID], f16, tag="lrs")
            nc.scalar.activation(lrs[:], ms[:], AF.Prelu, alpha=NEG)
            laws = sm.tile([P, HID], f16, tag="laws")
            nc.vector.tensor_tensor(out=laws[:], in0=lrs[:], in1=att_bc[:],
                                    op=OP.mult)
            lgs = sm.tile([P, heads], f32, tag="lgs")
            nc.vector.tensor_reduce(
                out=lgs[:],
                in_=laws[:].rearrange("p (h c) -> p h c", h=heads),
                axis=mybir.AxisListType.X, op=OP.add)
            gws = sm.tile([P, HID + heads], f16, tag="gws")
            nc.scalar.activation(gws[:, HID:HID + heads], lgs[:], AF.Exp)
            nc.vector.tensor_tensor(
                out=gws[:, 0:HID].rearrange("p (h c) -> p h c", h=heads),
                in0=xlw[:].rearrange("p (h c) -> p h c", h=heads),
                in1=gws[:, HID:HID + heads].unsqueeze(2)
                    .to_broadcast([P, heads, CW]),
                op=OP.mult)

            if pend is not None:
                part_b(*pend)
            pend = (w, s_t, gwd, gws)
        part_b(*pend)


def _emit_part_b(nc, *, w, s_t, gwd, gws, heads, h_out, bias_mat, c_, sm, agp,
                 T, CW, post_win):
    rows = slice(w * P, (w + 1) * P)
    agg = agp.tile([P, HID + heads], f32, tag="agg")
    for t in range(T):
        nc.tensor.matmul(agg[:], lhsT=s_t[:, t, :], rhs=gwd[:, t, :],
                         start=(t == 0), stop=False)
    nc.tensor.matmul(agg[:], lhsT=c_["ident"][:], rhs=gws[:],
                     start=False, stop=True)

    # normalize, bias, ELU
    den = sm.tile([P, heads], f32, tag="den")
    nc.vector.tensor_scalar_add(den[:], agg[:, HID:HID + heads], 1e-16)
    rden = sm.tile([P, heads], f32, tag="rden")
    nc.vector.reciprocal(rden[:], den[:])
    hn = sm.tile([P, HID], f32, tag="hn")
    for h in range(heads):
        nc.scalar.activation(hn[:, h * CW:(h + 1) * CW],
                             agg[:, h * CW:(h + 1) * CW], AF.Identity,
                             scale=rden[:, h:h + 1])
    hb = sm.tile([P, HID], f32, tag="hb")
    nc.vector.tensor_tensor(out=hb[:], in0=hn[:], in1=bias_mat[:], op=OP.add)
    # ELU(x) = relu(x) + exp(min(x, 0)) - 1
    # (tensor_tensor with broadcast consts: tensor_scalar runs a slow
    # DVE path, ~4us for a [128,256] op)
    mn = sm.tile([P, HID], f32, tag="mn")
    nc.vector.tensor_tensor(
        out=mn[:], in0=hb[:],
        in1=c_["zm1"][:, 0:1].to_broadcast([P, HID]), op=OP.min)
    ex = sm.tile([P, HID], f32, tag="ex")
    nc.scalar.activation(ex[:], mn[:], AF.Exp)
    el = sm.tile([P, HID], f32, tag="el")
    nc.vector.scalar_tensor_tensor(out=el[:], in0=hb[:], scalar=0.0,
                                   in1=ex[:], op0=OP.max, op1=OP.add)
    h_t = sm.tile([P, HID], f16, tag="h_t")
    nc.vector.tensor_tensor(
        out=h_t[:], in0=el[:],
        in1=c_["zm1"][:, 1:2].to_broadcast([P, HID]), op=OP.add)
    nc.scalar.dma_start(h_out[rows, :], h_t[:])
    if post_win is not None:
        post_win(w)


def kernel(**inputs):
    per_core, shared, batch_rows, ew = _preprocess(inputs)

    if ew not in _nc_cache:
        _nc_cache[ew] = _build(ew)
    nc = _nc_cache[ew]

    in_maps = []
    for c in range(NCORES):
        m = dict(shared)
        m.update(per_core[c])
        in_maps.append({k: np.ascontiguousarray(v) for k, v in m.items()})

    res = run_bass_kernel_spmd(nc, in_maps, core_ids=list(range(NCORES)))

    B = len(np.asarray(inputs["var_node_idx"]))
    out = np.zeros((B,), np.float32)
    for c in range(NCORES):
        rows = batch_rows[c]
        out[rows] = res.results[c]["out"][0, :len(rows)]
    return out


# revision 51
# speedup vs baseline: 1.0565x; 1.0565x over previous
"""Trainium2 Bass kernel for the 2-layer GATv2 + MLP-head model (nn_GAT_21028159881586).

Strategy (8 NeuronCores, SPMD single NEFF):
  * Destination-block partitioning: global nodes split into 8 slices of 3750
    (padded to 3840 = 30 windows x 128 per core).  Core c owns all edges whose
    destination lands in its slice, so segment softmax + aggregation are
    core-local.
  * Per layer: node transforms xl/xr = x@W+b on the local slice, AllGather of
    xl over the 8 cores, then 30 windows of 128 destinations each.
  * Per window (T tiles of 128 edges, single SWDGE gather):
      - ONE dma_gather of xl rows in (edge, channel) layout; trailing pad
        indices are -1 so the Q7 desc-gen trims them (desc-gen on the gpsimd
        pair is the kernel bottleneck at ~4.6ns/descriptor).
      - one-hot scatter matrix s_t[e,d] (DVE is_equal vs iota) for the
        aggregation matmul, and its transpose s_tT[d,e] (is_equal against a
        partition-broadcast of the dst-rel row) for expanding the window's
        128 xr rows to per-edge xr via PE matmul.
      - m = g + xr_e (DVE, PSUM+SBUF), leaky-relu (ACT Prelu),
        logits = rowwise (m*att).sum (DVE mult + reduce), alpha = exp(logit)
        (ACT), gw = g*alpha (DVE), and one accumulating PE matmul per tile
        with lhsT=s_t and rhs=[gw | alpha] producing both the aggregation and
        the softmax denominators in PSUM.
      - Pad edges carry dst-rel = -1 which matches no one-hot row/col, so
        they contribute nothing (no bias masking needed).
  * Softmax max-subtraction is skipped (logits are O(1); exp cannot
    overflow; mathematically identical).
  * MLP head: batch rows assigned to the core owning their var node; the
    selected h2 rows are dma_gathered transposed; 3-layer MLP runs
    transposed.

Everything runs in fp16 with fp32 PSUM accumulation.
"""

import numpy as np

import concourse.bacc as bacc
import concourse.tile as tile
import concourse.mybir as mybir
from concourse.bass_utils import run_bass_kernel_spmd

P = 128
NCORES = 8
N = 30000
NLOC_REAL = 3750          # real nodes per core
WIN = 30                  # destination windows per core
NLOC = WIN * P            # 3840 padded nodes per core
NALL = NCORES * NLOC      # 30720 padded global nodes
IN_DIM = 1281
KCH = 11                  # input-dim chunks of 128
KPAD = KCH * P            # 1408
HID = 256
HEADS1 = 4
BLOC = 640                # padded batch rows per core (actual max ~554)
NEG = 0.2
GE = 4                    # expansion-psum tiles per group (2 PSUM banks)
AGW = [30]                # AllGather chunk sizes in windows (single-writer rule
AGB = [0, 30]             # forbids >1 collective into one Shared tensor)
AGR = [w * P for w in AGB]            # chunk row boundaries (local)
AGOFF = [0]                           # global row offset of each chunk block
for _w in AGW:
    AGOFF.append(AGOFF[-1] + NCORES * _w * P)

PB = False                # build s_tT via partition-broadcast AP on DVE
TRIM = False              # -1-pad trimming of gather idxs

f32 = mybir.dt.float32
f16 = mybir.dt.float16
i16 = mybir.dt.int16
AF = mybir.ActivationFunctionType
OP = mybir.AluOpType

_nc_cache = {}


def _wrap16(idx2d: np.ndarray) -> np.ndarray:
    """(W, E) int -> (W*128, E//16) int16, wrapped in 16 partitions, replicated
    across the 8 gpsimd cores."""
    w, e = idx2d.shape
    assert e % 16 == 0
    t = idx2d.reshape(w, e // 16, 16).transpose(0, 2, 1)       # (W, 16, E/16)
    return np.tile(t, (1, 8, 1)).reshape(w * P, e // 16).astype(np.int16)


def _etile(v2d: np.ndarray) -> np.ndarray:
    """(W, E) -> (W*128, T) with [w*128+p, t] = v[w, t*128+p] (the layout of a
    transpose=False gather / per-tile PSUM partitions)."""
    w, e = v2d.shape
    t = v2d.reshape(w, e // P, P).transpose(0, 2, 1)           # (W, 128, T)
    return t.reshape(w * P, e // P)


def _preprocess(inputs):
    x = np.asarray(inputs["x"], np.float32)
    ei = np.asarray(inputs["edge_index"]).astype(np.int64)
    var_idx = np.asarray(inputs["var_node_idx"]).astype(np.int64)
    wt = np.asarray(inputs["wt_onehot"], np.float32)
    mut = np.asarray(inputs["mut_onehot"], np.float32)

    # self-loops are handled by a separate local path in the kernel (no
    # gather needed), so the edge lists carry only the real edges
    src = ei[0]
    dst = ei[1]

    # ---- degree-balanced window packing: per core, assign local nodes to
    # (window, slot) with LPT so every window has a near-equal edge count.
    import heapq
    perm_glob = np.zeros(N, np.int64)   # old global id -> core*NLOC + new local
    perm_tab = np.zeros(N, np.int64)    # old global id -> gather-table row
    perms = []
    dst_core_all = dst // NLOC_REAL
    dst_loc_all = dst - dst_core_all * NLOC_REAL
    for c in range(NCORES):
        deg = np.bincount(dst_loc_all[dst_core_all == c], minlength=NLOC_REAL)
        order_d = np.argsort(-deg, kind="stable")
        slots = [0] * WIN
        load = [0] * WIN
        perm = np.zeros(NLOC_REAL, np.int64)
        h = [(0, w) for w in range(WIN)]
        heapq.heapify(h)
        for v in order_d:
            while True:
                l, w = heapq.heappop(h)
                if slots[w] < P:
                    break
            perm[v] = w * P + slots[w]
            slots[w] += 1
            load[w] += int(deg[v])
            if slots[w] < P:
                heapq.heappush(h, (load[w], w))
        perms.append(perm)
        perm_glob[c * NLOC_REAL:(c + 1) * NLOC_REAL] = c * NLOC + perm
        # gather-table row under the block-chunked AllGather layout:
        # rows grouped as [chunk k][core r][local rows of chunk k]
        kk = np.searchsorted(np.asarray(AGR[1:]), perm, side="right")
        goff = np.asarray(AGOFF[:-1])[kk]
        szs = np.asarray([w * P for w in AGW])[kk]
        base = np.asarray(AGR[:-1])[kk]
        perm_tab[c * NLOC_REAL:(c + 1) * NLOC_REAL] = goff + c * szs + (perm - base)

    src_pad = perm_tab[src]
    dst_new = perm_glob[dst]

    order = np.argsort(dst_new, kind="stable")
    src_pad = src_pad[order]
    dst_s = dst_new[order]

    core_of = dst_s // NLOC
    dloc = dst_s - core_of * NLOC                           # local dst 0..3839
    win_of = dloc // P

    # max edges in any (core, window) -> uniform padded window size (x128)
    flat = core_of * WIN + win_of
    counts = np.bincount(flat, minlength=NCORES * WIN)
    ew = int(((counts.max() + P - 1) // P) * P)

    per_core = []
    for c in range(NCORES):
        sel = core_of == c
        sp_c, dl_c, w_c = src_pad[sel], dloc[sel], win_of[sel]
        srcw = np.zeros((WIN, ew), np.int64)
        drel = np.full((WIN, ew), -1.0, np.float32)
        for w in range(WIN):
            m = w_c == w
            k = int(m.sum())
            # order the window's edges by source for HBM locality
            o = np.argsort(sp_c[m], kind="stable")
            srcw[w, :k] = sp_c[m][o]
            if TRIM and w >= 2:
                # trailing -1 idxs are trimmed by the Q7 desc-gen; windows
                # 0/1 gather row 0 instead so the rotating SBUF buffers are
                # fully initialized with finite data.
                srcw[w, k:] = -1
            drel[w, :k] = (dl_c[m][o] - w * P).astype(np.float32)
        per_core.append(dict(
            si=_wrap16(srcw),
            drt=_etile(drel).astype(np.float32),
            drw=drel.astype(np.float16),
        ))

    # ---- shared weights / constants
    def pad_kT(w, m):  # (IN_DIM, m) -> (128, KCH*m) f16 chunked layout
        wp = np.zeros((KPAD, m), np.float32)
        wp[:IN_DIM] = w
        return wp.reshape(KCH, P, m).transpose(1, 0, 2).reshape(P, KCH * m).astype(np.float16)

    def two_chunk(w):  # (256, M) -> (128, 2*M) f16
        m = w.shape[1]
        return w.reshape(2, P, m).transpose(1, 0, 2).reshape(P, 2 * m).astype(np.float16)

    def rep_bias(b):  # (HID,) -> (128, HID) f32
        return np.broadcast_to(np.asarray(b, np.float32)[None, :], (P, HID)).copy()

    att1 = np.asarray(inputs["att1"], np.float32).reshape(1, HID)   # (4,64)->(1,256)
    att2 = np.asarray(inputs["att2"], np.float32).reshape(1, HID)

    hW1 = np.asarray(inputs["hW1"], np.float32)             # (296, 128)
    wlr1 = np.concatenate([np.asarray(inputs["Wl1"], np.float32),
                           np.asarray(inputs["Wr1"], np.float32)], axis=1)
    wlr2 = np.concatenate([np.asarray(inputs["Wl2"], np.float32),
                           np.asarray(inputs["Wr2"], np.float32)], axis=1)
    shared = dict(
        wlr1=pad_kT(wlr1, 2 * HID),
        wlr2=two_chunk(wlr2),
        attb1=np.broadcast_to(att1, (P, HID)).astype(np.float16).copy(),
        attb2=np.broadcast_to(att2, (P, HID)).astype(np.float16).copy(),
        blr1=np.concatenate([rep_bias(inputs["bl1"]), rep_bias(inputs["br1"])], 1),
        bias1=rep_bias(inputs["bias1"]),
        blr2=np.concatenate([rep_bias(inputs["bl2"]), rep_bias(inputs["br2"])], 1),
        bias2=rep_bias(inputs["bias2"]),
        hw1a=hW1[0:128].astype(np.float16),
        hw1b=hW1[128:256].astype(np.float16),
        hw1c=np.vstack([hW1[256:296], np.zeros((8, 128), np.float32)]).astype(np.float16),
        hw2=np.asarray(inputs["hW2"], np.float32).astype(np.float16),   # (128, 64)
        hw3=np.asarray(inputs["hW3"], np.float32).astype(np.float16),   # (64, 1)
        hb1=np.asarray(inputs["hb1"], np.float32).reshape(P, 1),
        hb2=np.asarray(inputs["hb2"], np.float32).reshape(64, 1),
        hb3=np.asarray(inputs["hb3"], np.float32).reshape(1, 1),
        iota=np.broadcast_to(np.arange(P, dtype=np.float32)[None, :], (P, P)).copy(),
        iotap=np.arange(P, dtype=np.float32).reshape(P, 1).copy(),
        ones1=np.ones((1, P), np.float16),
        ident=np.eye(P, dtype=np.float16),
        zm1=np.concatenate([np.zeros((P, 1), np.float32),
                            np.full((P, 1), -1.0, np.float32)], axis=1),
    )

    # ---- per-core x slices, transposed + padded, chunked layout (128, KCH*NLOC)
    for c in range(NCORES):
        xp = np.zeros((KPAD, NLOC), np.float32)
        xp[:IN_DIM, perms[c]] = x[c * NLOC_REAL:(c + 1) * NLOC_REAL].T
        per_core[c]["xt"] = xp.reshape(KCH, P, NLOC).transpose(1, 0, 2).reshape(
            P, KCH * NLOC).astype(np.float16)

    # ---- MLP batch assignment: rows go to the core owning their var node
    vcore = var_idx // NLOC_REAL
    vloc = perm_glob[var_idx] - vcore * NLOC
    batch_rows = []
    for c in range(NCORES):
        rows = np.nonzero(vcore == c)[0]
        assert len(rows) <= BLOC, f"core {c} has {len(rows)} batch rows > {BLOC}"
        batch_rows.append(rows)
        vi = np.zeros((1, BLOC), np.int64)
        vi[0, :len(rows)] = vloc[rows]
        per_core[c]["varloc"] = _wrap16(vi)
        wm = np.zeros((40, BLOC), np.float32)
        wm[:20, :len(rows)] = wt[rows].T
        wm[20:, :len(rows)] = mut[rows].T
        per_core[c]["wtmut"] = wm.astype(np.float16)

    return per_core, shared, batch_rows, ew


def _build(ew, no_collectives=False):
    T = ew // P
    nc = bacc.Bacc("TRN2", target_bir_lowering=False, debug=False,
                   num_devices=1 if no_collectives else NCORES,
                   num_swdge_queues=1)

    # ---------- I/O ----------
    io = {}
    io["xt"] = nc.dram_tensor("xt", [P, KCH * NLOC], f16, kind="ExternalInput")
    for nm, sh, dt in (
        ("wlr1", [P, KCH * 2 * HID], f16), ("wlr2", [P, 4 * HID], f16),
        ("attb1", [P, HID], f16), ("attb2", [P, HID], f16),
        ("blr1", [P, 2 * HID], f32), ("bias1", [P, HID], f32),
        ("blr2", [P, 2 * HID], f32), ("bias2", [P, HID], f32),
        ("hw1a", [P, P], f16), ("hw1b", [P, P], f16), ("hw1c", [48, P], f16),
        ("hw2", [P, 64], f16), ("hw3", [64, 1], f16),
        ("hb1", [P, 1], f32), ("hb2", [64, 1], f32), ("hb3", [1, 1], f32),
        ("iota", [P, P], f32), ("iotap", [P, 1], f32), ("ones1", [1, P], f16),
        ("ident", [P, P], f16), ("zm1", [P, 2], f32),
        ("si", [WIN * P, ew // 16], i16),
        ("drt", [WIN * P, T], f32),
        ("drw", [WIN, ew], f16),
        ("varloc", [P, BLOC // 16], i16), ("wtmut", [40, BLOC], f16),
    ):
        io[nm] = nc.dram_tensor(nm, sh, dt, kind="ExternalInput")
    out = nc.dram_tensor("out", [1, BLOC], f32, kind="ExternalOutput")

    with tile.TileContext(nc) as tc:
        with (
            tc.tile_pool(name="const", bufs=1) as cp,
            tc.tile_pool(name="dram", bufs=1, space="DRAM") as dr,
        ):
            # resident constants
            c_ = {}
            for nm in ("wlr2", "attb1", "attb2", "bias1", "blr2", "bias2",
                       "hw1a", "hw1b", "hw1c", "hw2",
                       "hw3", "hb1", "hb2", "hb3", "iota", "iotap", "ones1",
                       "ident", "zm1", "varloc", "wtmut"):
                h = io[nm]
                c_[nm] = cp.tile(list(h.shape), h.dtype, tag=nm, name=f"c_{nm}")
                nc.sync.dma_start(c_[nm][:], h[:])

            # DRAM scratch
            xl1_loc = dr.tile([NLOC, HID], f16)
            xr1_loc = dr.tile([NLOC, HID], f16)
            xl1_all = dr.tile([NALL, HID], f16, addr_space="Shared")
            h1_loc = dr.tile([NLOC, HID], f16)
            xl2_loc = dr.tile([NLOC, HID], f16)
            xr2_loc = dr.tile([NLOC, HID], f16)
            xl2_all = dr.tile([NALL, HID], f16, addr_space="Shared")
            h2_loc = dr.tile([NLOC, HID], f16)

            # ---------- phase A layer 1 ----------
            with (
                tc.tile_pool(name="pa_sb", bufs=2) as sb,
                tc.tile_pool(name="pa_xt", bufs=1) as xp,
                tc.tile_pool(name="pa_ps", bufs=4, space="PSUM") as ps,
            ):
                xt = xp.tile([P, KCH, NLOC], f16)
                nc.sync.dma_start(xt[:], io["xt"][:].rearrange("p (k n) -> p k n", k=KCH))
                wlr1 = xp.tile([P, KCH, 2 * HID], f16)
                nc.sync.dma_start(wlr1[:], io["wlr1"][:].rearrange("p (k n) -> p k n", k=KCH))
                blr1 = xp.tile([P, 2 * HID], f32)
                nc.sync.dma_start(blr1[:], io["blr1"][:])
                for nt in range(WIN):
                    pa = ps.tile([P, 2 * HID], f32, tag="pa")
                    for k in range(KCH):
                        nc.tensor.matmul(pa[:], lhsT=xt[:, k, nt * P:(nt + 1) * P],
                                         rhs=wlr1[:, k, :],
                                         start=(k == 0), stop=(k == KCH - 1))
                    o = sb.tile([P, 2 * HID], f16, tag="pao")
                    nc.vector.tensor_tensor(out=o[:], in0=pa[:], in1=blr1[:],
                                            op=OP.add)
                    nc.scalar.dma_start(xl1_loc[nt * P:(nt + 1) * P, :], o[:, 0:HID])
                    nc.scalar.dma_start(xr1_loc[nt * P:(nt + 1) * P, :], o[:, HID:2 * HID])

            def emit_ag(xloc, xall):
                if no_collectives:
                    nc.sync.dma_start(xall[0:NLOC, :], xloc[:])
                else:
                    nc.gpsimd.collective_compute(
                        "AllGather", OP.bypass,
                        replica_groups=[list(range(NCORES))],
                        ins=[xloc[:].opt()], outs=[xall[:].opt()])

            emit_ag(xl1_loc, xl1_all)

            # ---------- layer 1 message passing ----------
            _emit_layer(nc, tc, ew=ew, heads=HEADS1, xl_all=xl1_all,
                        xl_loc=xl1_loc, xr_loc=xr1_loc, h_out=h1_loc,
                        att_bc=c_["attb1"], bias_mat=c_["bias1"], io=io,
                        c_=c_, tag="l1")

            # ---------- phase A layer 2 ----------
            with (
                tc.tile_pool(name="pb_sb", bufs=2) as sb,
                tc.tile_pool(name="pb_ht", bufs=1) as hp,
                tc.tile_pool(name="pb_ps", bufs=4, space="PSUM") as ps,
            ):
                ht = hp.tile([P, 2, NLOC], f16)
                for k in range(2):
                    nc.sync.dma_start_transpose(ht[:, k, :],
                                                h1_loc[:, k * P:(k + 1) * P])
                blr2 = c_["blr2"]
                for nt in range(WIN):
                    pa = ps.tile([P, 2 * HID], f32, tag="pb")
                    for k in range(2):
                        nc.tensor.matmul(
                            pa[:], lhsT=ht[:, k, nt * P:(nt + 1) * P],
                            rhs=c_["wlr2"][:, k * 2 * HID:(k + 1) * 2 * HID],
                            start=(k == 0), stop=(k == 1))
                    o = sb.tile([P, 2 * HID], f16, tag="pbo")
                    nc.vector.tensor_tensor(out=o[:], in0=pa[:], in1=blr2[:],
                                            op=OP.add)
                    nc.scalar.dma_start(xl2_loc[nt * P:(nt + 1) * P, :], o[:, 0:HID])
                    nc.scalar.dma_start(xr2_loc[nt * P:(nt + 1) * P, :], o[:, HID:2 * HID])

            emit_ag(xl2_loc, xl2_all)

            # ---------- layer 2 message passing ----------
            _emit_layer(nc, tc, ew=ew, heads=1, xl_all=xl2_all,
                        xl_loc=xl2_loc, xr_loc=xr2_loc, h_out=h2_loc,
                        att_bc=c_["attb2"], bias_mat=c_["bias2"], io=io,
                        c_=c_, tag="l2")

            # ---------- MLP head ----------
            with (
                tc.tile_pool(name="mlp_sb", bufs=2) as sb,
                tc.tile_pool(name="mlp_ps", bufs=2, space="PSUM") as ps,
            ):
                sel = sb.tile([P, 2, BLOC], f16)
                nc.gpsimd.dma_gather(sel[:], h2_loc[:], c_["varloc"][:],
                                     num_idxs=BLOC, num_idxs_reg=BLOC,
                                     elem_size=HID, transpose=True)
                for c0, cn in ((0, 512), (512, BLOC - 512)):
                    z1p = ps.tile([P, 512], f32, tag="z1p")
                    nc.tensor.matmul(z1p[:, :cn], lhsT=c_["hw1a"][:],
                                     rhs=sel[:, 0, c0:c0 + cn], start=True, stop=False)
                    nc.tensor.matmul(z1p[:, :cn], lhsT=c_["hw1b"][:],
                                     rhs=sel[:, 1, c0:c0 + cn], start=False, stop=False)
                    nc.tensor.matmul(z1p[:, :cn], lhsT=c_["hw1c"][0:40, :],
                                     rhs=c_["wtmut"][:, c0:c0 + cn], start=False, stop=True)
                    z1 = sb.tile([P, 512], f16, tag="z1")
                    nc.scalar.activation(z1[:, :cn], z1p[:, :cn], AF.Relu,
                                         bias=c_["hb1"][:])
                    z2p = ps.tile([64, 512], f32, tag="z2p")
                    nc.tensor.matmul(z2p[:, :cn], lhsT=c_["hw2"][:],
                                     rhs=z1[:, :cn], start=True, stop=True)
                    z2 = sb.tile([64, 512], f16, tag="z2")
                    nc.scalar.activation(z2[:, :cn], z2p[:, :cn], AF.Relu,
                                         bias=c_["hb2"][:])
                    z3p = ps.tile([1, 512], f32, tag="z3p")
                    nc.tensor.matmul(z3p[:, :cn], lhsT=c_["hw3"][:],
                                     rhs=z2[:, :cn], start=True, stop=True)
                    z3 = sb.tile([1, 512], f32, tag="z3")
                    nc.scalar.activation(z3[:, :cn], z3p[:, :cn], AF.Identity,
                                         bias=c_["hb3"][:])
                    nc.sync.dma_start(out[0:1, c0:c0 + cn], z3[:, :cn])

    nc.compile()
    return nc


def _emit_layer(nc, tc, *, ew, heads, xl_all, xl_loc, xr_loc, h_out, att_bc,
                bias_mat, io, c_, tag, post_win=None):
    T = ew // P
    CW = HID // heads
    NG = (T + GE - 1) // GE
    NCH = (T + 7) // 8        # gather chunks of up to 8 tiles (1024 idxs max)
    regs = {}
    for c in range(NCH):
        tn = min(8, T - c * 8)
        if tn * P not in regs:
            regs[tn * P] = nc.gpsimd.to_reg(tn * P)
    import contextlib
    with contextlib.ExitStack() as st:
        bigp = st.enter_context(tc.tile_pool(name=f"{tag}_big", bufs=2))
        sm = st.enter_context(tc.tile_pool(name=f"{tag}_sm", bufs=3))
        mpp = st.enter_context(tc.tile_pool(name=f"{tag}_mp", bufs=2, space="PSUM"))
        agp = st.enter_context(tc.tile_pool(name=f"{tag}_ag", bufs=2, space="PSUM"))
        bcp = st.enter_context(tc.tile_pool(name=f"{tag}_bc", bufs=2, space="PSUM"))

        def part_b(w, s_t, gwd, gws):
            _emit_part_b(nc, w=w, s_t=s_t, gwd=gwd, gws=gws, heads=heads,
                         h_out=h_out, bias_mat=bias_mat, c_=c_, sm=sm, agp=agp,
                         T=T, CW=CW, post_win=post_win)

        pend = None
        for w in range(WIN):
            rows = slice(w * P, (w + 1) * P)
            si = sm.tile([P, ew // 16], i16, tag="si")
            nc.sync.dma_start(si[:], io["si"][rows, :])
            drt = sm.tile([P, T], f32, tag="drt")
            nc.sync.dma_start(drt[:], io["drt"][rows, :])
            drw = sm.tile([1, ew], f16, tag="drw")
            nc.sync.dma_start(drw[:], io["drw"][w:w + 1, :])
            xrw = sm.tile([P, HID], f16, tag="xrw")
            nc.sync.dma_start(xrw[:], xr_loc[rows, :])
            xlw = sm.tile([P, HID], f16, tag="xlw")
            nc.sync.dma_start(xlw[:], xl_loc[rows, :])

            # SWDGE gathers: xl rows of all window edges, (e, c) layout
            g = bigp.tile([P, T, HID], f16, tag="g", bufs=4)
            for c in range(NCH):
                tt0 = c * 8
                tn = min(8, T - tt0)
                nc.gpsimd.dma_gather(g[:, tt0:tt0 + tn, :], xl_all[:],
                                     si[:, tt0 * 8:(tt0 + tn) * 8],
                                     num_idxs=tn * P, num_idxs_reg=regs[tn * P],
                                     elem_size=HID, transpose=False)

            # one-hot scatter matrices
            s_t = bigp.tile([P, T, P], f16, tag="s_t")
            nc.vector.tensor_tensor(
                out=s_t[:],
                in0=drt[:].unsqueeze(2).to_broadcast([P, T, P]),
                in1=c_["iota"][:].unsqueeze(1).to_broadcast([P, T, P]),
                op=OP.is_equal)
            s_tT = bigp.tile([P, T, P], f16, tag="s_tT")
            s_tT_flat = s_tT[:].rearrange("p t e -> p (t e)")
            for gi in range((ew + 511) // 512):
                c0 = gi * 512
                cn = min(512, ew - c0)
                bc = bcp.tile([P, 512], f32, tag="bc")
                nc.tensor.matmul(bc[:, 0:cn], lhsT=c_["ones1"][:],
                                 rhs=drw[0:1, c0:c0 + cn], start=True, stop=True)
                nc.vector.tensor_tensor(
                    out=s_tT_flat[:, c0:c0 + cn],
                    in0=bc[:, 0:cn],
                    in1=c_["iotap"][:].to_broadcast([P, cn]),
                    op=OP.is_equal)

            # expand xr to per-edge via PE, add g (identity matmul), leaky-relu
            lr = bigp.tile([P, T, HID], f16, tag="lr")
            for gi in range(NG):
                t0 = gi * GE
                gn = min(GE, T - t0)
                mp = mpp.tile([P, GE, HID], f32, tag="mp")
                for j in range(gn):
                    nc.tensor.matmul(mp[:, j, :], lhsT=s_tT[:, t0 + j, :],
                                     rhs=xrw[:], start=True, stop=False)
                    nc.tensor.matmul(mp[:, j, :], lhsT=c_["ident"][:],
                                     rhs=g[:, t0 + j, :], start=False, stop=True)
                nc.scalar.activation(lr[:, t0:t0 + gn, :], mp[:, 0:gn, :],
                                     AF.Prelu, alpha=NEG)

            # logits = per-head dot with att (att_bc rows are the flattened
            # per-head att vector, so no head split is needed for the product)
            law = bigp.tile([P, T, HID], f16, tag="law")
            nc.vector.tensor_tensor(
                out=law[:], in0=lr[:],
                in1=att_bc[:].unsqueeze(1).to_broadcast([P, T, HID]),
                op=OP.mult)
            logit = sm.tile([P, T * heads], f32, tag="lg")
            nc.vector.tensor_reduce(
                out=logit[:],
                in_=law[:].rearrange("p t (h c) -> p (t h) c", h=heads),
                axis=mybir.AxisListType.X, op=OP.add)

            # alpha = exp(logit) straight into the den columns of gwd
            gwd = bigp.tile([P, T, HID + heads], f16, tag="gwd")
            nc.scalar.activation(
                gwd[:, :, HID:HID + heads],
                logit[:].rearrange("p (t h) -> p t h", h=heads), AF.Exp)
            nc.vector.tensor_tensor(
                out=gwd[:, :, 0:HID].rearrange("p t (h c) -> p t h c", h=heads),
                in0=g[:].rearrange("p t (h c) -> p t h c", h=heads),
                in1=gwd[:, :, HID:HID + heads].unsqueeze(3)
                    .to_broadcast([P, T, heads, CW]),
                op=OP.mult)

            # self-loop path: m_self = xl[d] + xr[d], all local, scattered
            # into the aggregation via an identity matmul in part B
            ms = sm.tile([P, HID], f16, tag="ms")
            nc.vector.tensor_tensor(out=ms[:], in0=xlw[:], in1=xrw[:],
                                    op=OP.add)
            lrs = sm.tile([P, HID], f16, tag="lrs")
            nc.scalar.activation(lrs[:], ms[:], AF.Prelu, alpha=NEG)
            laws = sm.tile([P, HID], f16, tag="laws")
            nc.vector.tensor_tensor(out=laws[:], in0=lrs[:], in1=att_bc[:],
                                    op=OP.mult)
            lgs = sm.tile([P, heads], f32, tag="lgs")
            nc.vector.tensor_reduce(
                out=lgs[:],
                in_=laws[:].rearrange("p (h c) -> p h c", h=heads),
                axis=mybir.AxisListType.X, op=OP.add)
            gws = sm.tile([P, HID + heads], f16, tag="gws")
            nc.scalar.activation(gws[:, HID:HID + heads], lgs[:], AF.Exp)
            nc.vector.tensor_tensor(
                out=gws[:, 0:HID].rearrange("p (h c) -> p h c", h=heads),
                in0=xlw[:].rearrange("p (h c) -> p h c", h=heads),
                in1=gws[:, HID:HID + heads].unsqueeze(2)
                    .to_broadcast([P, heads, CW]),
                op=OP.mult)

            if pend is not None:
                part_b(*pend)
            pend = (w, s_t, gwd, gws)
        part_b(*pend)


def _emit_part_b(nc, *, w, s_t, gwd, gws, heads, h_out, bias_mat, c_, sm, agp,
                 T, CW, post_win):
    rows = slice(w * P, (w + 1) * P)
    agg = agp.tile([P, HID + heads], f32, tag="agg")
    for t in range(T):
        nc.tensor.matmul(agg[:], lhsT=s_t[:, t, :], rhs=gwd[:, t, :],
                         start=(t == 0), stop=False)
    nc.tensor.matmul(agg[:], lhsT=c_["ident"][:], rhs=gws[:],
                     start=False, stop=True)

    # normalize, bias, ELU
    den = sm.tile([P, heads], f32, tag="den")
    nc.vector.tensor_scalar_add(den[:], agg[:, HID:HID + heads], 1e-16)
    rden = sm.tile([P, heads], f32, tag="rden")
    nc.vector.reciprocal(rden[:], den[:])
    hn = sm.tile([P, HID], f32, tag="hn")
    for h in range(heads):
        nc.scalar.activation(hn[:, h * CW:(h + 1) * CW],
                             agg[:, h * CW:(h + 1) * CW], AF.Identity,
                             scale=rden[:, h:h + 1])
    hb = sm.tile([P, HID], f32, tag="hb")
    nc.vector.tensor_tensor(out=hb[:], in0=hn[:], in1=bias_mat[:], op=OP.add)
    # ELU(x) = relu(x) + exp(min(x, 0)) - 1
    # (tensor_tensor with broadcast consts: tensor_scalar runs a slow
    # DVE path, ~4us for a [128,256] op)
    mn = sm.tile([P, HID], f32, tag="mn")
    nc.vector.tensor_tensor(
        out=mn[:], in0=hb[:],
        in1=c_["zm1"][:, 0:1].to_broadcast([P, HID]), op=OP.min)
    ex = sm.tile([P, HID], f32, tag="ex")
    nc.scalar.activation(ex[:], mn[:], AF.Exp)
    el = sm.tile([P, HID], f32, tag="el")
    nc.vector.scalar_tensor_tensor(out=el[:], in0=hb[:], scalar=0.0,
                                   in1=ex[:], op0=OP.max, op1=OP.add)
    h_t = sm.tile([P, HID], f16, tag="h_t")
    nc.vector.tensor_tensor(
        out=h_t[:], in0=el[:],
        in1=c_["zm1"][:, 1:2].to_broadcast([P, HID]), op=OP.add)
    nc.scalar.dma_start(h_out[rows, :], h_t[:])
    if post_win is not None:
        post_win(w)


def kernel(**inputs):
    per_core, shared, batch_rows, ew = _preprocess(inputs)

    if ew not in _nc_cache:
        _nc_cache[ew] = _build(ew)
    nc = _nc_cache[ew]

    in_maps = []
    for c in range(NCORES):
        m = dict(shared)
        m.update(per_core[c])
        in_maps.append({k: np.ascontiguousarray(v) for k, v in m.items()})

    res = run_bass_kernel_spmd(nc, in_maps, core_ids=list(range(NCORES)))

    B = len(np.asarray(inputs["var_node_idx"]))
    out = np.zeros((B,), np.float32)
    for c in range(NCORES):
        rows = batch_rows[c]
        out[rows] = res.results[c]["out"][0, :len(rows)]
    return out
